# revision 39
# baseline (speedup 1.0000x reference)
"""Bass/Tile TRN2 kernel for nn_BilateralCostVolume — windowed-gather design.

out[b,r,h,w] = <bilinear(f2n, p + d_r), bilinear(f1n, p - d_r)> * mask
with d_r = BM + (du,dv), r = dv*9+du, du/dv in linspace(-4,4,9), t=0.5.

Key idea: for each pixel, the 81 displacement samples of one warp all lie in
an 11x11 window around a per-pixel center (BM enters the center; du/dv are
integer-ish shifts).  Gather that window ONCE per (pixel, warp) — 11 rows of
12px x 64c bf16 (1536 B descriptors) from a padded, edge-replicated,
channel-last table — then do separable interpolation shared across
displacements:

  x-stage: XI[du, py, c] = sum_t wx4[du,t] * win[py, du+t, c]   (4 taps)
  y-stage: FW[dv, du, c] = sum_k wy3[dv,k] * XI[du, dv+k, c]    (3 taps)
  dot:     out[r] = sum_c FWF * FWB  (warp B du-axis pre-flipped), * mask

Weights / gather indices / masks are host-precomputed from BM (f32 math
mirroring the reference).  Tables are built on device: l2-normalize over c,
transpose to [y, x, c], pad, cast bf16.

Sharding: 160 (b, h) pixel rows over 8 cores; 128 pixels of a row on SBUF
partitions; each core sees only its batch's features (host slices).
"""

import numpy as np
from ml_dtypes import bfloat16

import concourse.bass as bass
import concourse.bacc as bacc
import concourse.mybir as mybir
import concourse.tile as tile
from concourse import bass_utils
from concourse.masks import make_identity

MD = 4
R = 81
B, C, H, W = 2, 48, 80, 128
SW = np.float32(W) / np.float32(W - 1)
SH = np.float32(H) / np.float32(H - 1)
CP = 64                  # padded channels in tables
XPAD = 12
YPAD = 12
XT = W + 2 * XPAD + 2    # 154 (even)
YT = H + 2 * YPAD        # 104
NPAIR = XT // 2          # 77
NROWS = YT * NPAIR       # 8008
NROWS_AL = 8016          # allocated rows (tail pad for 768-elem overrun)
NCORES = 8
GPC = 20                 # (b, h) groups per core
NW = 11                  # window rows per pixel
EL = 768                 # gather elem_size (12 px * 64 c, bf16 -> 1536 B)
ES = 128                 # gather elem_step (2 px * 64 c = 256 B)
NIDX = NW * 128          # 1408 idxs per gather

F32 = mybir.dt.float32
I16 = mybir.dt.int16
BF16 = mybir.dt.bfloat16
AF = mybir.ActivationFunctionType
OP = mybir.AluOpType

LIN = np.linspace(-MD, MD, 2 * MD + 1).astype(np.float32)


# ------------------------------------------------------------------ program
def _overlap_ap(t_ap, offset_elems=0):
    """View a [NROWS_AL, ES] dram tile as overlapped gather rows
    [[ES, NROWS_AL], [1, EL]]."""
    a = t_ap.copy()
    v = a.ap
    v.clear()
    v.extend([(ES, NROWS_AL - 5), (1, EL)])
    a.offset = a.offset + offset_elems
    return a


def build_program(dbg=False):
    nc = bacc.Bacc(
        "TRN2",
        target_bir_lowering=False,
        debug=False,
        enable_asserts=False,
        num_devices=NCORES,
        num_swdge_queues=2,
    )

    f2b_d = nc.dram_tensor("f2b", [C, H, W], F32, kind="ExternalInput")
    f1b_d = nc.dram_tensor("f1b", [C, H, W], F32, kind="ExternalInput")
    wxy_d = nc.dram_tensor("wxy", [GPC, 128, 126], F32, kind="ExternalInput")
    msk_d = nc.dram_tensor("msk", [GPC, 128, R], F32, kind="ExternalInput")
    gidx_d = nc.dram_tensor("gidx", [GPC, 2, 128, 88], I16,
                            kind="ExternalInput")
    out_d = nc.dram_tensor("out", [GPC, 128, R], F32, kind="ExternalOutput")

    with tile.TileContext(nc) as tc:
        with (
            tc.tile_pool(name="const", bufs=1) as constp,
            tc.tile_pool(name="dram", bufs=1, space="DRAM") as dramp,
        ):
            ident = constp.tile([128, 128], F32)
            make_identity(nc, ident[:])

            tabF = dramp.tile([NROWS_AL, ES], BF16)   # f2n table (warp F)
            tabB = dramp.tile([NROWS_AL, ES], BF16)   # f1n table (warp B)

            # ---------------- Phase 1: normalize + padded tables ----------
            # single pool + per-plane tags so the two planes pipeline
            with (
                tc.tile_pool(name="p1", bufs=1) as p1,
                tc.tile_pool(name="ps1", bufs=2, space="PSUM") as ps1,
            ):
                for pi_, (src_d, tab) in enumerate(
                        ((f2b_d, tabF), (f1b_d, tabB))):
                    fc = p1.tile([C, H * W], F32, tag=f"fc{pi_}")
                    srcv = src_d.ap().rearrange("c h w -> c (h w)")
                    HW2 = H * W // 2
                    nc.sync.dma_start(out=fc[:, 0:HW2], in_=srcv[:, 0:HW2])
                    nc.sync.dma_start(out=fc[:, HW2:], in_=srcv[:, HW2:])
                    T = p1.tile([128, H, C], F32, tag=f"T{pi_}")
                    sq = p1.tile([128, H, C], F32, tag=f"sq{pi_}")
                    ssq = p1.tile([128, H], F32, tag=f"ssq{pi_}")
                    tb = p1.tile([128, H, C], BF16, tag=f"tb{pi_}")
                    for hb in range(8):
                        hs = slice(hb * 10, (hb + 1) * 10)
                        pt = ps1.tile([128, 10 * C], F32, tag=f"pt{pi_}")
                        for j in range(10):
                            h = hb * 10 + j
                            nc.tensor.transpose(
                                out=pt[:, j * C:(j + 1) * C],
                                in_=fc[:, h * W:(h + 1) * W],
                                identity=ident[:C, :C])
                        nc.scalar.copy(T[:, hs, :], pt[:])
                        nc.scalar.square(sq[:, hs, :], T[:, hs, :])
                        nc.vector.tensor_reduce(
                            ssq[:, hs], sq[:, hs, :],
                            axis=mybir.AxisListType.X, op=OP.add)
                        nc.vector.tensor_copy(tb[:, hs, :], T[:, hs, :])
                    rn = p1.tile([128, H], F32, tag=f"rn{pi_}")
                    nc.scalar.activation(rn[:], ssq[:], AF.Sqrt,
                                         bias=np.float32(1e-6))
                    nc.vector.reciprocal(rn[:], rn[:])
                    re = p1.tile([128, H, C], BF16, tag=f"re{pi_}")
                    nc.vector.tensor_copy(
                        re[:], rn[:].unsqueeze(-1).broadcast_to([128, H, C]))
                    tn = p1.tile([128, H, C], BF16, tag=f"tn{pi_}")
                    nc.vector.tensor_mul(tn[:], tb[:], re[:])

                    # interior write: px (XPAD + w) of row (YPAD + h)
                    dst = tab[:].copy()
                    v = dst.ap
                    v.clear()
                    # dims: (w 128 part-ish? no — DMA from SBUF [128,...]
                    # source partitions = w); dst elem offset:
                    # ((YPAD+h)*XT + XPAD + w)*CP + c
                    v.extend([(CP, 128), (XT * CP, H), (1, C)])
                    dst.offset = dst.offset + (YPAD * XT + XPAD) * CP
                    nc.sync.dma_start(out=dst, in_=tn[:])

                    # x pads: left cols [0, XPAD) <- col x=0 ; right
                    # [XPAD+W, XT) <- col x=W-1  (dram->dram, bcast px)
                    for px0, npx, srcx in ((0, XPAD, 0),
                                           (XPAD + W, XT - XPAD - W, W - 1)):
                        sap = tab[:].copy()
                        v = sap.ap
                        v.clear()
                        v.extend([(XT * CP, YT - 24), (0, npx), (1, CP)])
                        sap.offset = (sap.offset
                                      + (YPAD * XT + XPAD + srcx) * CP)
                        dap = tab[:].copy()
                        v = dap.ap
                        v.clear()
                        v.extend([(XT * CP, YT - 24), (CP, npx), (1, CP)])
                        dap.offset = dap.offset + (YPAD * XT + px0) * CP
                        nc.sync.dma_start(out=dap, in_=sap)

                    # y pads: rows [0, YPAD) <- row y=0 ; [YPAD+H, YT) <- last
                    for y0, ny, srcy in ((0, YPAD, YPAD),
                                         (YPAD + H, YT - YPAD - H,
                                          YPAD + H - 1)):
                        sap = tab[:].copy()
                        v = sap.ap
                        v.clear()
                        v.extend([(0, ny), (1, XT * CP)])
                        sap.offset = sap.offset + srcy * XT * CP
                        dap = tab[:].copy()
                        v = dap.ap
                        v.clear()
                        v.extend([(XT * CP, ny), (1, XT * CP)])
                        dap.offset = dap.offset + y0 * XT * CP
                        nc.sync.dma_start(out=dap, in_=sap)

            # ---------------- Phase 2: per-group windows ------------------
            # Software-pipelined emission: loads/gathers of group g+2 and
            # mults of group g+1 are emitted before the adds/dot of group g
            # so in-order engine queues never stall on cross-engine deps.
            with (
                tc.tile_pool(name="win", bufs=2) as winp,
                tc.tile_pool(name="ld", bufs=4) as ldp,
                tc.tile_pool(name="cmp", bufs=2) as cmp_,
                tc.tile_pool(name="yst", bufs=1) as yst,
            ):
                XM_ENG = {(0, 0): "v", (0, 1): "a", (0, 2): "a",
                          (0, 3): "a", (1, 0): "a", (1, 1): "p",
                          (1, 2): "p", (1, 3): "v"}
                YM_ENG = {(0, 0): "v", (0, 1): "v", (0, 2): "a",
                          (1, 0): "a", (1, 1): "p", (1, 2): "p"}
                # ramp maps: spread mult batches evenly while the pipeline
                # fills (DVE idle in the head) / drains (Act idle in tail)
                XM_RAMP = {(0, 0): "v", (0, 1): "a", (0, 2): "p",
                           (0, 3): "v", (1, 0): "a", (1, 1): "p",
                           (1, 2): "v", (1, 3): "a"}
                YM_RAMP = {(0, 0): "v", (0, 1): "a", (0, 2): "p",
                           (1, 0): "v", (1, 1): "a", (1, 2): "p"}
                YM_TAIL = {(0, 0): "a", (0, 1): "p", (0, 2): "a",
                           (1, 0): "a", (1, 1): "p", (1, 2): "a"}

                def mul_op(eng, out, in0, sc):
                    if eng == "a":
                        nc.scalar.mul(out, in0, sc)
                    elif eng == "p":
                        nc.gpsimd.tensor_scalar(
                            out=out, in0=in0, scalar1=sc, scalar2=None,
                            op0=OP.mult)
                    else:
                        nc.vector.tensor_scalar(
                            out=out, in0=in0, scalar1=sc, scalar2=None,
                            op0=OP.mult)

                tiles = {}

                def S0(g):
                    d = {}
                    d["wv"] = ldp.tile([128, 126], F32, tag="wv", name="wv")
                    nc.sync.dma_start(out=d["wv"][:], in_=wxy_d.ap()[g])
                    d["mk"] = ldp.tile([128, R], F32, tag="mk", name="mk")
                    nc.sync.dma_start(out=d["mk"][:], in_=msk_d.ap()[g])
                    d["wins"] = []
                    for wi, tab in enumerate((tabF, tabB)):
                        gx = ldp.tile([128, 88], I16, tag=f"gx{wi}",
                                      name=f"gx{wi}")
                        nc.sync.dma_start(out=gx[:], in_=gidx_d.ap()[g, wi])
                        win = winp.tile([128, NW, EL], BF16, tag=f"win{wi}",
                                        name=f"win{wi}")
                        nc.gpsimd.dma_gather(
                            out_ap=win[:],
                            in_ap=_overlap_ap(tab[:]),
                            idxs_ap=gx[:],
                            num_idxs=NIDX,
                            num_idxs_reg=NIDX,
                            elem_size=EL,
                            elem_step=ES,
                            single_packet=False,
                            queue_num=wi,
                        )
                        d["wins"].append(win)
                    d["TMPS"] = {}
                    d["XIs"] = {}
                    d["FWs"] = {}
                    tiles[g] = d

                def SM(g, wi):
                    """Tap-product multiplies for warp wi of group g."""
                    xm = XM_RAMP if g < 2 else XM_ENG
                    d = tiles[g]
                    wv = d["wv"]
                    win = d["wins"][wi]
                    TMPS = [cmp_.tile([128, 9, NW, C], BF16, tag=f"TMP{j}",
                                      name=f"TMP{j}") for j in range(3)]
                    d["TMPS"][wi] = TMPS
                    XI = cmp_.tile([128, 9, NW, C], BF16, tag=f"XI{wi}",
                                   name=f"XI{wi}")
                    d["XIs"][wi] = XI
                    wb = 63 * wi
                    for t in range(4):
                        dstt = XI if t == 0 else TMPS[t - 1]
                        eng = xm[(wi, t)]
                        for dui in range(9):
                            do = dui if wi == 0 else 8 - dui
                            mul_op(
                                eng, dstt[:, do, :, :],
                                win[:, :, (dui + t) * CP:(dui + t) * CP + C],
                                wv[:, wb + dui * 4 + t:wb + dui * 4 + t + 1])

                def SD(g, wi):
                    """x-adds for warp wi of group g (DVE)."""
                    d = tiles[g]
                    XI = d["XIs"][wi]
                    TMPS = d["TMPS"][wi]
                    nc.vector.tensor_add(TMPS[0][:], TMPS[0][:], TMPS[1][:])
                    nc.vector.tensor_add(XI[:], XI[:], TMPS[2][:])
                    nc.vector.tensor_add(XI[:], XI[:], TMPS[0][:])

                def SBW(g, wi):
                    """y-stage for warp wi of group g."""
                    ym = (YM_RAMP if g < 2 else
                          YM_TAIL if g >= GPC - 2 else YM_ENG)
                    d = tiles[g]
                    wv = d["wv"]
                    XI = d["XIs"][wi]
                    YTS = [yst.tile([128, 9, 9, C], BF16, tag=f"YT{j}",
                                    name=f"YT{j}") for j in range(2)]
                    FW = yst.tile([128, 9, 9, C], BF16, tag=f"FW{wi}",
                                  name=f"FW{wi}")
                    d["FWs"][wi] = FW
                    wb = 63 * wi + 36
                    for k in range(3):
                        dstt = FW if k == 0 else YTS[k - 1]
                        eng = ym[(wi, k)]
                        for dvi in range(9):
                            mul_op(
                                eng, dstt[:, dvi, :, :],
                                XI[:, :, dvi + k, :],
                                wv[:, wb + dvi * 3 + k:wb + dvi * 3 + k + 1])
                    nc.vector.tensor_add(FW[:], FW[:], YTS[0][:])
                    nc.vector.tensor_add(FW[:], FW[:], YTS[1][:])

                def SE(g):
                    """dot + tree + mask + store for group g."""
                    d = tiles.pop(g)
                    FWF, FWB = d["FWs"][0], d["FWs"][1]
                    nc.vector.tensor_mul(FWF[:], FWF[:], FWB[:])
                    P = FWF[:].rearrange("p a b c -> p (a b) c")
                    wdt = C
                    while wdt > 3:
                        nc.vector.tensor_add(
                            P[:, :, 0:wdt // 2], P[:, :, 0:wdt // 2],
                            P[:, :, wdt // 2:wdt])
                        wdt //= 2
                    ot = yst.tile([128, R], F32, tag="ot")
                    nc.vector.tensor_reduce(
                        ot[:], P[:, :, 0:3], axis=mybir.AxisListType.X,
                        op=OP.add)
                    nc.gpsimd.tensor_mul(ot[:], ot[:], d["mk"][:])
                    nc.sync.dma_start(out=out_d.ap()[g], in_=ot[:])

                # software pipeline: mults of g+1 are emitted around the
                # y-stage/dot of g so no engine queue head-blocks.
                S0(0)
                S0(1)
                SM(0, 0)
                SD(0, 0)
                SM(0, 1)
                SD(0, 1)
                for g in range(GPC):
                    if g + 1 < GPC:
                        SM(g + 1, 0)
                    SBW(g, 0)
                    if g + 1 < GPC:
                        SD(g + 1, 0)
                        SM(g + 1, 1)
                    SBW(g, 1)
                    SE(g)
                    if g + 1 < GPC:
                        SD(g + 1, 1)
                    if g + 2 < GPC:
                        S0(g + 2)

    nc.compile()
    return nc


# ------------------------------------------------------------------ host
def _host_fields(BM, sign, b):
    """Window geometry + separable weights + mask for one warp.
    Mirrors reference f32 math. Returns arrays indexed [h, w]."""
    BMx = BM[b, 0].astype(np.float32)
    BMy = BM[b, 1].astype(np.float32)
    x = np.arange(W, dtype=np.float32)[None, :]
    y = np.arange(H, dtype=np.float32)[:, None]
    s = np.float32(sign)
    ix = (SW * (x[:, :, None] + s * (BMx[:, :, None] + LIN[None, None, :]))
          - np.float32(0.5))
    iy = (SH * (y[:, :, None] + s * (BMy[:, :, None] + LIN[None, None, :]))
          - np.float32(0.5))
    x0f = np.floor(ix)
    y0f = np.floor(iy)
    fx = (ix - x0f).astype(np.float32)
    fy = (iy - y0f).astype(np.float32)
    x0 = x0f.astype(np.int32)
    y0 = y0f.astype(np.int32)

    basex = SW * (x + s * BMx) - np.float32(0.5)
    basey = SH * (y + s * BMy) - np.float32(0.5)
    cx = np.floor(basex + 0.5).astype(np.int32)
    cy = np.floor(basey + 0.5).astype(np.int32)

    xstart = cx - 5 + XPAD
    pair = xstart >> 1
    sL = np.round(s * LIN).astype(np.int32)[None, None, :]
    e_x = x0 - (cx[:, :, None] + sL)
    assert e_x.min() >= -1 and e_x.max() <= 0, (e_x.min(), e_x.max())
    pi = (xstart - 2 * pair)[:, :, None]
    t0 = pi + e_x + 1
    hh, ww, rr = np.meshgrid(np.arange(H), np.arange(W), np.arange(9),
                             indexing="ij")
    qq = rr if sign > 0 else 8 - rr
    wx4 = np.zeros((H, W, 9, 4), np.float32)
    wx4[hh, ww, qq, t0] = 1.0 - fx
    wx4[hh, ww, qq, t0 + 1] = fx

    e_y = y0 - (cy[:, :, None] + sL)
    assert e_y.min() >= -1 and e_y.max() <= 0, (e_y.min(), e_y.max())
    wy3 = np.zeros((H, W, 9, 3), np.float32)
    if sign > 0:
        wy3[hh, ww, rr, e_y + 1] = 1.0 - fy
        wy3[hh, ww, rr, e_y + 2] = fy
        idx0 = (cy - 5 + YPAD) * NPAIR + pair
        idxstep = NPAIR
    else:
        wy3[hh, ww, rr, 1 - e_y] = 1.0 - fy
        wy3[hh, ww, rr, -e_y] = fy
        idx0 = (cy + 5 + YPAD) * NPAIR + pair
        idxstep = -NPAIR
    rlo = idx0 + (10 * idxstep if idxstep < 0 else 0)
    rhi = idx0 + (10 * idxstep if idxstep > 0 else 0)
    assert rlo.min() >= 0 and rhi.max() < NROWS, (rlo.min(), rhi.max())
    assert xstart.min() >= 0 and (2 * pair + 12).max() <= XT

    inbx = ((x0 >= 0) & (x0 <= W - 1)).astype(np.float32)
    inbx1 = ((x0 + 1 >= 0) & (x0 + 1 <= W - 1)).astype(np.float32)
    inby = ((y0 >= 0) & (y0 <= H - 1)).astype(np.float32)
    inby1 = ((y0 + 1 >= 0) & (y0 + 1 <= H - 1)).astype(np.float32)
    mx = (1 - fx) * inbx + fx * inbx1
    my = (1 - fy) * inby + fy * inby1
    m2 = mx[:, :, None, :] * my[:, :, :, None]        # [H, W, dv, du]
    mbin = np.where(m2 < np.float32(0.999), np.float32(0), np.float32(1))
    return dict(wx4=wx4, wy3=wy3, idx0=idx0, idxstep=idxstep, mask=mbin)


def make_in_maps(feature1, feature2, BM):
    f1 = np.ascontiguousarray(np.asarray(feature1, dtype=np.float32))
    f2 = np.ascontiguousarray(np.asarray(feature2, dtype=np.float32))
    bm = np.asarray(BM, dtype=np.float32)

    fields = {}
    for b in range(B):
        fields[(b, +1)] = _host_fields(bm, +1, b)
        fields[(b, -1)] = _host_fields(bm, -1, b)

    in_maps = []
    groups_per_core = []
    for k in range(NCORES):
        gs = list(range(GPC * k, GPC * (k + 1)))
        groups_per_core.append(gs)
        b0 = gs[0] // H
        assert all(g // H == b0 for g in gs)
        wxy = np.zeros((GPC, 128, 126), np.float32)
        msk = np.zeros((GPC, 128, R), np.float32)
        gidx = np.zeros((GPC, 2, 128, 88), np.int16)
        for gi, g in enumerate(gs):
            h = g % H
            for wi, sign in enumerate((+1, -1)):
                fl = fields[(b0, sign)]
                wxy[gi, :, 63 * wi:63 * wi + 36] = \
                    fl["wx4"][h].reshape(128, 36)
                wxy[gi, :, 63 * wi + 36:63 * wi + 63] = \
                    fl["wy3"][h].reshape(128, 27)
                rows = (fl["idx0"][h][None, :]
                        + np.arange(NW)[:, None] * fl["idxstep"])  # [11, 128]
                wrapped = rows.reshape(-1).astype(np.int16)
                wrapped = wrapped.reshape(88, 16).T      # [16, 88]
                gidx[gi, wi] = np.tile(wrapped, (8, 1))
            m = (fields[(b0, +1)]["mask"][h]
                 * fields[(b0, -1)]["mask"][h])          # [W, dv, du]
            msk[gi] = m.reshape(128, R)
        in_maps.append({
            "f2b": f2[b0], "f1b": f1[b0],
            "wxy": wxy, "msk": msk, "gidx": gidx,
        })
    return in_maps, groups_per_core, None


_NC_CACHE = {}


def get_program():
    if "nc" not in _NC_CACHE:
        _NC_CACHE["nc"] = build_program()
    return _NC_CACHE["nc"]


def assemble_output(results, groups_per_core, _unused=None):
    out = np.zeros((B, R, H, W), np.float32)
    for k in range(NCORES):
        core_out = results[k]["out"]          # [GPC, 128, R]
        for gi, g in enumerate(groups_per_core[k]):
            b, h = g // H, g % H
            out[b, :, h, :] = core_out[gi].T
    return out


def kernel(feature1, feature2, BM):
    nc = get_program()
    in_maps, groups_per_core, _ = make_in_maps(feature1, feature2, BM)
    res = bass_utils.run_bass_kernel_spmd(
        nc, in_maps, core_ids=list(range(NCORES)))
    return assemble_output(res.results, groups_per_core)
